# revision 6
# baseline (speedup 1.0000x reference)
"""Sparse attention (ProbSparse-style) Trainium2 Bass kernel.

Problem (per batch element b, data-parallel over 8 NeuronCores):
  Q = x @ Wq.T ; K = x @ Wk.T ; V = x @ Wv.T            [L=2048, D=512]
  QK_sample[l,s] = Q[l] . K[index_sample[l,s]]           [L, 40]
  M[l] = max_s QK_sample - sum_s QK_sample / L
  sel = top40(M)  (as a set; the reference scatter makes order irrelevant)
  scores = Q[sel] @ K.T / sqrt(D); attn = softmax(scores)
  ctx = broadcast(mean(V)); ctx[sel] = attn @ V

Numerics strategy (top-40 boundary gaps are as small as 0.02 in M):
  - K and V are computed with a 3-term bf16x2 split matmul
    (xh*wh + xl*wh + xh*wl, host-split halves) -> ~1e-5 absolute error,
    fp32-class, at full bf16 PE rate.
  - Approx M for ALL rows uses bf16 Q and bf16 K (error sigma ~0.2),
    extracted from per-chunk S = Q K^T PSUM blocks with fused
    tensor_tensor_reduce against a shipped u8 sample mask
    (multiply-mask max is safe: sampled max > 0 w.p. 1-2^-40;
    dup-count correction is deferred to the exact stage).
  - Candidates = { M_approx >= approx-top40 - DELTA }, DELTA=1.5 covers
    ~8 sigma; measured rank-40 to rank-64 M gap is 2.5-4.8 so the
    candidate count stays well under the 128-slot budget.
  - Exact stage on <= 128 candidate rows: gather x rows from DRAM
    (indirect DMA), exact fp32 Q_cand, exact S_cand vs the fp32-class K,
    TTR with gathered u8 mask+count rows -> exact M_cand -> exact top-40
    threshold -> softmax over S_cand -> upd = attn @ V -> indirect
    scatter of the 40 selected rows into ctx (bounds_check skips the
    rest).

kernel(**inputs) accepts the FULL inputs and returns the FULL
[8, 2048, 512] f32 output; batch is sharded over 8 cores.
"""

import math

import numpy as np
import ml_dtypes

import concourse.bacc as bacc
import concourse.bass as bass
import concourse.mybir as mybir
import concourse.tile as tile
from concourse.bass_utils import run_bass_kernel_spmd
from concourse.masks import make_identity

P = 128
L = 2048
D = 512
B = 8
NL = L // P        # 16 query chunks
ND = D // P        # 4 feature chunks
NJ = L // 512      # 4 key blocks of 512
NT = 40
SCALE = 1.0 / math.sqrt(D)
DELTA = 1.5        # candidate band below approx T40
NEG = -3.0e38
SKIP_IDX = 99999.0  # scatter index sentinel (> bounds_check -> row skipped)

f32 = mybir.dt.float32
bf16 = mybir.dt.bfloat16
u8 = mybir.dt.uint8
i32 = mybir.dt.int32
u32 = mybir.dt.uint32
AX = mybir.AxisListType
OP = mybir.AluOpType
ACTF = mybir.ActivationFunctionType


def build():
    nc = bacc.Bacc("TRN2", target_bir_lowering=False)

    x_d = nc.dram_tensor("x_nat", [L, D], f32, kind="ExternalInput")
    xth_d = nc.dram_tensor("xTh", [D, L], bf16, kind="ExternalInput")
    xtl_d = nc.dram_tensor("xTl", [D, L], bf16, kind="ExternalInput")
    xm_d = nc.dram_tensor("xmeanT", [D, 1], f32, kind="ExternalInput")
    wqh_d = nc.dram_tensor("wqTh", [D, D], bf16, kind="ExternalInput")
    wkh_d = nc.dram_tensor("wkTh", [D, D], bf16, kind="ExternalInput")
    wkl_d = nc.dram_tensor("wkTl", [D, D], bf16, kind="ExternalInput")
    wvh_d = nc.dram_tensor("wvTh", [D, D], bf16, kind="ExternalInput")
    wvl_d = nc.dram_tensor("wvTl", [D, D], bf16, kind="ExternalInput")
    wq_d = nc.dram_tensor("wqT", [D, D], f32, kind="ExternalInput")
    wv_d = nc.dram_tensor("wvT", [D, D], f32, kind="ExternalInput")
    mask_d = nc.dram_tensor("mask01", [L, L], u8, kind="ExternalInput")
    cnt_d = nc.dram_tensor("countf", [L, L], u8, kind="ExternalInput")
    ctx_d = nc.dram_tensor("ctx", [L, D], f32, kind="ExternalOutput")

    with tile.TileContext(nc) as tc:
        with (
            tc.tile_pool(name="const", bufs=1) as cst,
            tc.tile_pool(name="proj", bufs=1) as proj,       # KT/KTb/QTb/V resident
            tc.tile_pool(name="mstuff", bufs=1) as mst,      # M / topk / sel smalls
            tc.tile_pool(name="mstream", bufs=3) as mstr,    # mask chunks
            tc.tile_pool(name="scr", bufs=3) as scr,         # TTR scratch
            tc.tile_pool(name="acc", bufs=2) as accp,        # per-chunk accums
            tc.tile_pool(name="cand", bufs=1) as cnd,        # exact-stage tiles
            tc.tile_pool(name="ps", bufs=4, space="PSUM") as ps,
            tc.tile_pool(name="ps_s", bufs=4, space="PSUM") as ps_s,  # S_cand (held)
            tc.tile_pool(name="dram", bufs=1, space="DRAM") as drp,
        ):
            # ---------------- constants ----------------
            ident = cst.tile([P, P], f32, tag="ident")
            make_identity(nc, ident[:])
            ones_r1 = cst.tile([1, P], f32, tag="ones_r1")
            nc.vector.memset(ones_r1[:], 1.0)
            negone = cst.tile([P, 1], f32, tag="negone")
            nc.vector.memset(negone[:], -1.0)
            negbig = cst.tile([P, 1], f32, tag="negbig")
            nc.vector.memset(negbig[:], NEG)
            big9 = cst.tile([P, 1], f32, tag="big9")
            nc.vector.memset(big9[:], SKIP_IDX)
            qidx_i = cst.tile([P, 16], i32, tag="qidx_i")     # value p + 128*c
            nc.gpsimd.iota(qidx_i[:], pattern=[[P, 16]], base=0, channel_multiplier=1)
            qidx_f = cst.tile([P, 16], f32, tag="qidx_f")
            nc.vector.tensor_copy(qidx_f[:], qidx_i[:])

            # resident projection outputs
            KT = [proj.tile([P, L], f32, tag=f"KT{ic}", name=f"KT{ic}") for ic in range(ND)]
            KTb = [proj.tile([P, L], bf16, tag=f"KTb{ic}", name=f"KTb{ic}") for ic in range(ND)]
            QTb = [proj.tile([P, L], bf16, tag=f"QTb{ic}", name=f"QTb{ic}") for ic in range(ND)]
            V = [proj.tile([P, D], f32, tag=f"V{jc}", name=f"V{jc}") for jc in range(NL)]

            with tc.tile_pool(name="xw", bufs=1) as xw:
                # ---------------- phase 0: loads ----------------
                xTh = [xw.tile([P, L], bf16, tag=f"xTh{dc}", name=f"xTh{dc}") for dc in range(ND)]
                xTl = [xw.tile([P, L], bf16, tag=f"xTl{dc}", name=f"xTl{dc}") for dc in range(ND)]
                wqh = [xw.tile([P, D], bf16, tag=f"wqh{dc}", name=f"wqh{dc}") for dc in range(ND)]
                wkh = [xw.tile([P, D], bf16, tag=f"wkh{dc}", name=f"wkh{dc}") for dc in range(ND)]
                wkl = [xw.tile([P, D], bf16, tag=f"wkl{dc}", name=f"wkl{dc}") for dc in range(ND)]
                wvh = [xw.tile([P, D], bf16, tag=f"wvh{dc}", name=f"wvh{dc}") for dc in range(ND)]
                wvl = [xw.tile([P, D], bf16, tag=f"wvl{dc}", name=f"wvl{dc}") for dc in range(ND)]
                wqT = [xw.tile([P, D], f32, tag=f"wqT{dc}", name=f"wqT{dc}") for dc in range(ND)]
                wvT = [xw.tile([P, D], f32, tag=f"wvT{dc}", name=f"wvT{dc}") for dc in range(ND)]
                xmT = [xw.tile([P, 1], f32, tag=f"xmT{dc}", name=f"xmT{dc}") for dc in range(ND)]
                for dc in range(ND):
                    sl = slice(dc * P, (dc + 1) * P)
                    nc.sync.dma_start(xTh[dc][:], xth_d[sl, :])
                    nc.sync.dma_start(xTl[dc][:], xtl_d[sl, :])
                    nc.sync.dma_start(wqh[dc][:], wqh_d[sl, :])
                    nc.sync.dma_start(wkh[dc][:], wkh_d[sl, :])
                    nc.sync.dma_start(wkl[dc][:], wkl_d[sl, :])
                    nc.sync.dma_start(wvh[dc][:], wvh_d[sl, :])
                    nc.sync.dma_start(wvl[dc][:], wvl_d[sl, :])
                    nc.sync.dma_start(wqT[dc][:], wq_d[sl, :])
                    nc.sync.dma_start(wvT[dc][:], wv_d[sl, :])
                    nc.sync.dma_start(xmT[dc][:], xm_d[sl, :])

                # ---------------- phase 1: projections ----------------
                # K: 3-term bf16x2 (fp32-class), into KT f32 + KTb bf16
                for ic in range(ND):
                    isl = slice(ic * P, (ic + 1) * P)
                    for jb in range(NJ):
                        jsl = slice(jb * 512, (jb + 1) * 512)
                        pk = ps.tile([P, 512], f32, tag="blk")
                        n = 0
                        for dc in range(ND):
                            for lh, rh in (
                                (wkh[dc][:, isl], xTh[dc][:, jsl]),
                                (wkh[dc][:, isl], xTl[dc][:, jsl]),
                                (wkl[dc][:, isl], xTh[dc][:, jsl]),
                            ):
                                nc.tensor.matmul(
                                    pk[:], lh, rh,
                                    start=(n == 0), stop=(n == 3 * ND - 1),
                                )
                                n += 1
                        nc.scalar.copy(KT[ic][:, jsl], pk[:])
                        nc.vector.tensor_copy(KTb[ic][:, jsl], pk[:])

                # Q approx: single bf16 term
                for ic in range(ND):
                    isl = slice(ic * P, (ic + 1) * P)
                    for jb in range(NJ):
                        jsl = slice(jb * 512, (jb + 1) * 512)
                        pq = ps.tile([P, 512], f32, tag="blk")
                        for dc in range(ND):
                            nc.tensor.matmul(
                                pq[:], wqh[dc][:, isl], xTh[dc][:, jsl],
                                start=(dc == 0), stop=(dc == ND - 1),
                            )
                        nc.scalar.copy(QTb[ic][:, jsl], pq[:])

                # V: 3-term bf16x2
                for jc in range(NL):
                    jsl = slice(jc * P, (jc + 1) * P)
                    pv = ps.tile([P, 512], f32, tag="blk")
                    n = 0
                    for dc in range(ND):
                        for lh, rh in (
                            (xTh[dc][:, jsl], wvh[dc][:]),
                            (xTl[dc][:, jsl], wvh[dc][:]),
                            (xTh[dc][:, jsl], wvl[dc][:]),
                        ):
                            nc.tensor.matmul(
                                pv[:], lh, rh,
                                start=(n == 0), stop=(n == 3 * ND - 1),
                            )
                            n += 1
                    nc.scalar.copy(V[jc][:], pv[:])

                # Vmean = xmeanT.T @ WvT (exact fp32), then broadcast + write ctx
                pvm = ps.tile([1, 512], f32, tag="blk")
                for dc in range(ND):
                    nc.tensor.matmul(
                        pvm[:1, :], xmT[dc][:], wvT[dc][:],
                        start=(dc == 0), stop=(dc == ND - 1),
                    )
                vmean = mst.tile([1, 512], f32, tag="vmean")
                nc.scalar.copy(vmean[:], pvm[:1, :])
                pvb = ps.tile([P, 512], f32, tag="blk")
                nc.tensor.matmul(pvb[:], ones_r1[:], vmean[:], start=True, stop=True)
                vmean_bc = mst.tile([P, 512], f32, tag="vmean_bc")
                nc.vector.tensor_copy(vmean_bc[:], pvb[:])
                for jc in range(NL):
                    nc.sync.dma_start(ctx_d[jc * P : (jc + 1) * P, :], vmean_bc[:])

                # ---------------- phase 2: approx M (bf16 S) ----------------
                M_all = mst.tile([P, 16], f32, tag="M_all")
                for lc in range(NL):
                    lsl = slice(lc * P, (lc + 1) * P)
                    mk = mstr.tile([P, L], u8, tag="mk")
                    nc.sync.dma_start(mk[:], mask_d[lsl, :])
                    amax = accp.tile([P, NJ], f32, tag="amax")
                    asum = accp.tile([P, NJ], f32, tag="asum")
                    for jb in range(NJ):
                        jsl = slice(jb * 512, (jb + 1) * 512)
                        pss = ps.tile([P, 512], f32, tag="blk")
                        for ic in range(ND):
                            nc.tensor.matmul(
                                pss[:], QTb[ic][:, lsl], KTb[ic][:, jsl],
                                start=(ic == 0), stop=(ic == ND - 1),
                            )
                        s1 = scr.tile([P, 512], f32, tag="scrt")
                        nc.vector.tensor_tensor(
                            out=s1[:], in0=pss[:], in1=mk[:, jsl], op=OP.mult
                        )
                        nc.vector.reduce_max(
                            amax[:, jb : jb + 1], s1[:], axis=AX.X
                        )
                        nc.vector.reduce_sum(
                            asum[:, jb : jb + 1], s1[:], axis=AX.X
                        )
                    t1 = accp.tile([P, 1], f32, tag="t1")
                    t2 = accp.tile([P, 1], f32, tag="t2")
                    nc.vector.reduce_max(t1[:], amax[:], axis=AX.X)
                    nc.vector.reduce_sum(t2[:], asum[:], axis=AX.X)
                    nc.vector.tensor_scalar_mul(t2[:], t2[:], -1.0 / L)
                    nc.vector.tensor_tensor(
                        out=M_all[:, lc : lc + 1], in0=t1[:], in1=t2[:], op=OP.add
                    )

                # ---------------- phase 3: approx top-40 -> candidates ------
                pmt = ps.tile([16, P], f32, tag="blk")
                nc.tensor.transpose(pmt[:16, :P], M_all[:], ident[:])
                work = mst.tile([16, P], f32, tag="work")
                nc.vector.tensor_copy(work[:], pmt[:16, :P])
                cand40 = mst.tile([16, NT], f32, tag="cand40")
                for r in range(5):
                    nc.vector.max(out=cand40[:, 8 * r : 8 * r + 8], in_=work[:])
                    if r < 4:
                        nc.vector.match_replace(
                            out=work[:], in_to_replace=cand40[:, 8 * r : 8 * r + 8],
                            in_values=work[:], imm_value=NEG,
                        )
                lin_dram = drp.tile([16 * NT], f32, tag="lin_dram")
                nc.sync.dma_start(lin_dram[:], cand40[:])
                lin = mst.tile([1, 16 * NT], f32, tag="lin")
                nc.sync.dma_start(lin[:], lin_dram[:].rearrange("(a b) -> a b", a=1))
                top40 = mst.tile([1, NT], f32, tag="top40")
                for r in range(5):
                    nc.vector.max(out=top40[:, 8 * r : 8 * r + 8], in_=lin[:])
                    if r < 4:
                        nc.vector.match_replace(
                            out=lin[:], in_to_replace=top40[:, 8 * r : 8 * r + 8],
                            in_values=lin[:], imm_value=NEG,
                        )
                t40d = mst.tile([1, 1], f32, tag="t40d")
                nc.vector.tensor_scalar_add(t40d[:], top40[:, NT - 1 : NT], -DELTA)
                ptb = ps.tile([P, 1], f32, tag="blk")
                nc.tensor.matmul(ptb[:P, :1], ones_r1[:], t40d[:], start=True, stop=True)
                tbc = mst.tile([P, 1], f32, tag="tbc")
                nc.vector.tensor_copy(tbc[:], ptb[:P, :1])

                selmask = mst.tile([P, 16], u8, tag="selmask")
                nc.vector.tensor_tensor(
                    out=selmask[:], in0=M_all[:], in1=tbc[:].to_broadcast([P, 16]),
                    op=OP.is_ge,
                )
                midx = mst.tile([P, 16], f32, tag="midx")
                nc.vector.tensor_copy(midx[:], negone[:].to_broadcast([P, 16]))
                nc.vector.copy_predicated(midx[:], selmask[:], qidx_f[:])

                wrap_in = mst.tile([16, P], f32, tag="wrap_in")
                nc.sync.dma_start(wrap_in[:], midx[:])
                spg = mst.tile([16, 8], f32, tag="spg")
                nfound = mst.tile([1, 1], u32, tag="nfound")
                nc.gpsimd.sparse_gather(out=spg[:], in_=wrap_in[:], num_found=nfound[:])
                spg_cl = mst.tile([16, 8], f32, tag="spg_cl")
                nc.vector.tensor_scalar_max(spg_cl[:], spg[:], 0.0)
                nc.vector.tensor_scalar_min(spg_cl[:], spg_cl[:], float(L - 1))

                cand_dram = drp.tile([P], f32, tag="cand_dram")
                nc.sync.dma_start(
                    cand_dram[:].rearrange("(f p) -> p f", p=16), spg_cl[:]
                )
                candq_f = mst.tile([P, 1], f32, tag="candq_f")
                nc.sync.dma_start(candq_f[:], cand_dram[:].rearrange("l -> l ()"))
                candq_i = mst.tile([P, 1], i32, tag="candq_i")
                nc.vector.tensor_copy(candq_i[:], candq_f[:])

                nf_f = mst.tile([1, 1], f32, tag="nf_f")
                nc.vector.tensor_copy(nf_f[:], nfound[:])
                pnb = ps.tile([P, 1], f32, tag="blk")
                nc.tensor.matmul(pnb[:P, :1], ones_r1[:], nf_f[:], start=True, stop=True)
                nbc = mst.tile([P, 1], f32, tag="nbc")
                nc.vector.tensor_copy(nbc[:], pnb[:P, :1])
                invalid = mst.tile([P, 1], u8, tag="invalid")
                nc.vector.tensor_tensor(
                    out=invalid[:], in0=qidx_f[:, 0:1], in1=nbc[:], op=OP.is_ge
                )

                # ---------------- phase 4a: exact candidates ----------------
                x_cand = cnd.tile([P, D], f32, tag="x_cand")
                nc.gpsimd.indirect_dma_start(
                    out=x_cand[:], out_offset=None, in_=x_d[:],
                    in_offset=bass.IndirectOffsetOnAxis(ap=candq_i[:, :1], axis=0),
                )
                xcT = [cnd.tile([P, P], f32, tag=f"xcT{dc}", name=f"xcT{dc}") for dc in range(ND)]
                for dc in range(ND):
                    pxc = ps.tile([P, P], f32, tag="blk")
                    nc.tensor.transpose(
                        pxc[:P, :P], x_cand[:, dc * P : (dc + 1) * P], ident[:]
                    )
                    nc.vector.tensor_copy(xcT[dc][:], pxc[:P, :P])

                QcT = [cnd.tile([P, P], f32, tag=f"QcT{ic}", name=f"QcT{ic}") for ic in range(ND)]
                for ic in range(ND):
                    isl = slice(ic * P, (ic + 1) * P)
                    pqc = ps.tile([P, P], f32, tag="blk")
                    for dc in range(ND):
                        nc.tensor.matmul(
                            pqc[:P, :P], wqT[dc][:, isl], xcT[dc][:],
                            start=(dc == 0), stop=(dc == ND - 1),
                        )
                    nc.vector.tensor_copy(QcT[ic][:], pqc[:P, :P])

                gm = cnd.tile([P, L], u8, tag="gm")
                nc.gpsimd.indirect_dma_start(
                    out=gm[:], out_offset=None, in_=mask_d[:],
                    in_offset=bass.IndirectOffsetOnAxis(ap=candq_i[:, :1], axis=0),
                )
                gc = cnd.tile([P, L], u8, tag="gc")
                nc.gpsimd.indirect_dma_start(
                    out=gc[:], out_offset=None, in_=cnt_d[:],
                    in_offset=bass.IndirectOffsetOnAxis(ap=candq_i[:, :1], axis=0),
                )

                psS = []
                cmax = cnd.tile([P, NJ], f32, tag="cmax")
                csum = cnd.tile([P, NJ], f32, tag="csum")
                for jb in range(NJ):
                    jsl = slice(jb * 512, (jb + 1) * 512)
                    pss2 = ps_s.tile([P, 512], f32, tag="psSc")
                    psS.append(pss2)
                    for ic in range(ND):
                        nc.tensor.matmul(
                            pss2[:], QcT[ic][:], KT[ic][:, jsl],
                            start=(ic == 0), stop=(ic == ND - 1),
                        )
                    s3 = scr.tile([P, 512], f32, tag="scrt")
                    nc.vector.tensor_tensor(
                        out=s3[:], in0=pss2[:], in1=gm[:, jsl], op=OP.mult
                    )
                    nc.vector.reduce_max(cmax[:, jb : jb + 1], s3[:], axis=AX.X)
                    s4 = scr.tile([P, 512], f32, tag="scrt")
                    nc.vector.scalar_tensor_tensor(
                        out=s4[:], in0=pss2[:], scalar=-1.0 / L, in1=gc[:, jsl],
                        op0=OP.mult, op1=OP.mult,
                        accum_out=csum[:, jb : jb + 1],
                    )
                u1 = cnd.tile([P, 1], f32, tag="u1")
                u2 = cnd.tile([P, 1], f32, tag="u2")
                M_cand = cnd.tile([P, 1], f32, tag="M_cand")
                nc.vector.reduce_max(u1[:], cmax[:], axis=AX.X)
                nc.vector.reduce_sum(u2[:], csum[:], axis=AX.X)
                nc.vector.tensor_tensor(out=M_cand[:], in0=u1[:], in1=u2[:], op=OP.add)
                nc.vector.copy_predicated(M_cand[:], invalid[:], negbig[:])

                # exact top-40 threshold among candidates
                pmc = ps.tile([1, P], f32, tag="blk")
                nc.tensor.transpose(pmc[:1, :P], M_cand[:], ident[:])
                mcT = cnd.tile([1, P], f32, tag="mcT")
                nc.vector.tensor_copy(mcT[:], pmc[:1, :P])
                etop = cnd.tile([1, NT], f32, tag="etop")
                for r in range(5):
                    nc.vector.max(out=etop[:, 8 * r : 8 * r + 8], in_=mcT[:])
                    if r < 4:
                        nc.vector.match_replace(
                            out=mcT[:], in_to_replace=etop[:, 8 * r : 8 * r + 8],
                            in_values=mcT[:], imm_value=NEG,
                        )
                pte = ps.tile([P, 1], f32, tag="blk")
                nc.tensor.matmul(
                    pte[:P, :1], ones_r1[:], etop[:, NT - 1 : NT], start=True, stop=True
                )
                tebc = cnd.tile([P, 1], f32, tag="tebc")
                nc.vector.tensor_copy(tebc[:], pte[:P, :1])
                sel2 = cnd.tile([P, 1], u8, tag="sel2")
                nc.vector.tensor_tensor(
                    out=sel2[:], in0=M_cand[:], in1=tebc[:], op=OP.is_ge
                )
                scat_f = cnd.tile([P, 1], f32, tag="scat_f")
                nc.vector.tensor_copy(scat_f[:], big9[:])
                nc.vector.copy_predicated(scat_f[:], sel2[:], candq_f[:])
                scat_i = cnd.tile([P, 1], i32, tag="scat_i")
                nc.vector.tensor_copy(scat_i[:], scat_f[:])

            # xTh/xTl/weights freed here
            with tc.tile_pool(name="expp", bufs=1) as expp:
                # ---------------- phase 4b: softmax + update ----------------
                rmax4 = expp.tile([P, NJ], f32, tag="rmax4")
                for jb in range(NJ):
                    nc.vector.reduce_max(rmax4[:, jb : jb + 1], psS[jb][:], axis=AX.X)
                rmax = expp.tile([P, 1], f32, tag="rmax")
                nc.vector.reduce_max(rmax[:], rmax4[:], axis=AX.X)
                negb = expp.tile([P, 1], f32, tag="negb")
                nc.vector.tensor_scalar_mul(negb[:], rmax[:], -SCALE)
                exp_sb = expp.tile([P, L], f32, tag="exp_sb")
                sume4 = expp.tile([P, NJ], f32, tag="sume4")
                for jb in range(NJ):
                    jsl = slice(jb * 512, (jb + 1) * 512)
                    nc.scalar.activation(
                        out=exp_sb[:, jsl], in_=psS[jb][:], func=ACTF.Exp,
                        bias=negb[:], scale=SCALE,
                        accum_out=sume4[:, jb : jb + 1],
                    )
                sume = expp.tile([P, 1], f32, tag="sume")
                nc.vector.reduce_sum(sume[:], sume4[:], axis=AX.X)
                recip = expp.tile([P, 1], f32, tag="recip")
                nc.vector.reciprocal(recip[:], sume[:])

                expT = [expp.tile([P, P], f32, tag=f"expT{jc}", name=f"expT{jc}") for jc in range(NL)]
                for jc in range(NL):
                    pet = ps.tile([P, P], f32, tag="blk")
                    nc.tensor.transpose(
                        pet[:P, :P], exp_sb[:, jc * P : (jc + 1) * P], ident[:]
                    )
                    nc.vector.tensor_copy(expT[jc][:], pet[:P, :P])

                pu = ps.tile([P, 512], f32, tag="blk")
                for jc in range(NL):
                    nc.tensor.matmul(
                        pu[:], expT[jc][:], V[jc][:],
                        start=(jc == 0), stop=(jc == NL - 1),
                    )
                upd = expp.tile([P, D], f32, tag="upd")
                nc.scalar.activation(
                    out=upd[:], in_=pu[:], func=ACTF.Copy, bias=0.0, scale=recip[:]
                )
                nc.gpsimd.indirect_dma_start(
                    out=ctx_d[:],
                    out_offset=bass.IndirectOffsetOnAxis(ap=scat_i[:, :1], axis=0),
                    in_=upd[:], in_offset=None,
                    bounds_check=L - 1, oob_is_err=False,
                )

    nc.compile()
    return nc


_NC = None


def _get_nc():
    global _NC
    if _NC is None:
        _NC = build()
    return _NC


def _split_bf16(a):
    hi = a.astype(ml_dtypes.bfloat16)
    lo = (a - hi.astype(np.float32)).astype(ml_dtypes.bfloat16)
    return hi, lo


def _host_prep(x, Wq, Wk, Wv, index_sample):
    x = np.asarray(x, dtype=np.float32)
    Wq = np.asarray(Wq, dtype=np.float32)
    Wk = np.asarray(Wk, dtype=np.float32)
    Wv = np.asarray(Wv, dtype=np.float32)
    idx = np.asarray(index_sample)

    wqT = np.ascontiguousarray(Wq.T)
    wvT = np.ascontiguousarray(Wv.T)
    wqh, _ = _split_bf16(wqT)
    wkh, wkl = _split_bf16(np.ascontiguousarray(Wk.T))
    wvh, wvl = _split_bf16(wvT)

    rows = np.arange(L)[:, None]
    mask01 = np.zeros((L, L), dtype=np.uint8)
    mask01[rows, idx] = 1
    countf = np.zeros((L, L), dtype=np.uint8)
    np.add.at(countf, (rows, idx), 1)

    shared = {
        "wqTh": wqh, "wkTh": wkh, "wkTl": wkl, "wvTh": wvh, "wvTl": wvl,
        "wqT": wqT, "wvT": wvT, "mask01": mask01, "countf": countf,
    }
    in_maps = []
    for b in range(B):
        xb = np.ascontiguousarray(x[b])
        xT = np.ascontiguousarray(xb.T)
        xth, xtl = _split_bf16(xT)
        xmean = (xb.astype(np.float64).mean(axis=0) / 1.0).astype(np.float32)
        in_maps.append(
            {
                "x_nat": xb,
                "xTh": xth,
                "xTl": xtl,
                "xmeanT": xmean.reshape(D, 1),
                **shared,
            }
        )
    return in_maps


def kernel(x, Wq, Wk, Wv, index_sample, _trace=False, _result_box=None):
    in_maps = _host_prep(x, Wq, Wk, Wv, index_sample)
    nc = _get_nc()
    res = run_bass_kernel_spmd(nc, in_maps, core_ids=list(range(B)), trace=_trace)
    if _result_box is not None:
        _result_box.append(res)
    out = np.stack([np.asarray(res.results[b]["ctx"]) for b in range(B)], axis=0)
    return out


# revision 7
# speedup vs baseline: 1.1281x; 1.1281x over previous
"""Sparse attention (ProbSparse-style) Trainium2 Bass kernel.

Problem (per batch element b, data-parallel over 8 NeuronCores):
  Q = x @ Wq.T ; K = x @ Wk.T ; V = x @ Wv.T            [L=2048, D=512]
  QK_sample[l,s] = Q[l] . K[index_sample[l,s]]           [L, 40]
  M[l] = max_s QK_sample - sum_s QK_sample / L
  sel = top40(M)  (as a set; the reference scatter makes order irrelevant)
  scores = Q[sel] @ K.T / sqrt(D); attn = softmax(scores)
  ctx = broadcast(mean(V)); ctx[sel] = attn @ V

Numerics strategy (top-40 boundary gaps are as small as 0.02 in M):
  - K and V are computed with a 3-term bf16x2 split matmul
    (xh*wh + xl*wh + xh*wl, host-split halves) -> ~1e-5 absolute error,
    fp32-class, at full bf16 PE rate.
  - Approx M for ALL rows uses bf16 Q and bf16 K (error sigma ~0.2),
    extracted from per-chunk S = Q K^T PSUM blocks with fused
    tensor_tensor_reduce against a shipped u8 sample mask
    (multiply-mask max is safe: sampled max > 0 w.p. 1-2^-40;
    dup-count correction is deferred to the exact stage).
  - Candidates = { M_approx >= approx-top40 - DELTA }, DELTA=1.5 covers
    ~8 sigma; measured rank-40 to rank-64 M gap is 2.5-4.8 so the
    candidate count stays well under the 128-slot budget.
  - Exact stage on <= 128 candidate rows: gather x rows from DRAM
    (indirect DMA), exact fp32 Q_cand, exact S_cand vs the fp32-class K,
    TTR with gathered u8 mask+count rows -> exact M_cand -> exact top-40
    threshold -> softmax over S_cand -> upd = attn @ V -> indirect
    scatter of the 40 selected rows into ctx (bounds_check skips the
    rest).

kernel(**inputs) accepts the FULL inputs and returns the FULL
[8, 2048, 512] f32 output; batch is sharded over 8 cores.
"""

import math

import numpy as np
import ml_dtypes

import concourse.bacc as bacc
import concourse.bass as bass
import concourse.mybir as mybir
import concourse.tile as tile
from concourse.bass_utils import run_bass_kernel_spmd
from concourse.masks import make_identity

P = 128
L = 2048
D = 512
B = 8
NL = L // P        # 16 query chunks
ND = D // P        # 4 feature chunks
NJ = L // 512      # 4 key blocks of 512
NT = 40
SCALE = 1.0 / math.sqrt(D)
DELTA = 1.5        # candidate band below approx T40
NEG = -3.0e38
SKIP_IDX = 99999.0  # scatter index sentinel (> bounds_check -> row skipped)

f32 = mybir.dt.float32
bf16 = mybir.dt.bfloat16
u8 = mybir.dt.uint8
i32 = mybir.dt.int32
u32 = mybir.dt.uint32
AX = mybir.AxisListType
OP = mybir.AluOpType
ACTF = mybir.ActivationFunctionType


def build():
    nc = bacc.Bacc("TRN2", target_bir_lowering=False)

    x_d = nc.dram_tensor("x_nat", [L, D], f32, kind="ExternalInput")
    xth_d = nc.dram_tensor("xTh", [D, L], bf16, kind="ExternalInput")
    xtl_d = nc.dram_tensor("xTl", [D, L], bf16, kind="ExternalInput")
    xm_d = nc.dram_tensor("xmeanT", [D, 1], f32, kind="ExternalInput")
    wqh_d = nc.dram_tensor("wqTh", [D, D], bf16, kind="ExternalInput")
    wkh_d = nc.dram_tensor("wkTh", [D, D], bf16, kind="ExternalInput")
    wkl_d = nc.dram_tensor("wkTl", [D, D], bf16, kind="ExternalInput")
    wvh_d = nc.dram_tensor("wvTh", [D, D], bf16, kind="ExternalInput")
    wvl_d = nc.dram_tensor("wvTl", [D, D], bf16, kind="ExternalInput")
    wq_d = nc.dram_tensor("wqT", [D, D], f32, kind="ExternalInput")
    wv_d = nc.dram_tensor("wvT", [D, D], f32, kind="ExternalInput")
    mask_d = nc.dram_tensor("mask01", [L, L], u8, kind="ExternalInput")
    cnt_d = nc.dram_tensor("countf", [L, L], u8, kind="ExternalInput")
    ctx_d = nc.dram_tensor("ctx", [L, D], f32, kind="ExternalOutput")

    with tile.TileContext(nc) as tc:
        with (
            tc.tile_pool(name="const", bufs=1) as cst,
            tc.tile_pool(name="proj", bufs=1) as proj,       # KT/KTb/QTb/V resident
            tc.tile_pool(name="mstuff", bufs=1) as mst,      # M / topk / sel smalls
            tc.tile_pool(name="mstream", bufs=3) as mstr,    # mask chunks
            tc.tile_pool(name="scr", bufs=3) as scr,         # TTR scratch
            tc.tile_pool(name="acc", bufs=2) as accp,        # per-chunk accums
            tc.tile_pool(name="cand", bufs=1) as cnd,        # exact-stage tiles
            tc.tile_pool(name="ps", bufs=3, space="PSUM") as ps,
            tc.tile_pool(name="ps_s", bufs=4, space="PSUM") as ps_s,  # S_cand (held)
            tc.tile_pool(name="dram", bufs=1, space="DRAM") as drp,
        ):
            # ---------------- constants ----------------
            ident = cst.tile([P, P], f32, tag="ident")
            make_identity(nc, ident[:])
            ones_r1 = cst.tile([1, P], f32, tag="ones_r1")
            nc.vector.memset(ones_r1[:], 1.0)
            negone = cst.tile([P, 1], f32, tag="negone")
            nc.vector.memset(negone[:], -1.0)
            negbig = cst.tile([P, 1], f32, tag="negbig")
            nc.vector.memset(negbig[:], NEG)
            big9 = cst.tile([P, 1], f32, tag="big9")
            nc.vector.memset(big9[:], SKIP_IDX)
            qidx_i = cst.tile([P, 16], i32, tag="qidx_i")     # value p + 128*c
            nc.gpsimd.iota(qidx_i[:], pattern=[[P, 16]], base=0, channel_multiplier=1)
            qidx_f = cst.tile([P, 16], f32, tag="qidx_f")
            nc.vector.tensor_copy(qidx_f[:], qidx_i[:])

            # resident projection outputs
            KT = [proj.tile([P, L], f32, tag=f"KT{ic}", name=f"KT{ic}") for ic in range(ND)]
            KTb = [proj.tile([P, L], bf16, tag=f"KTb{ic}", name=f"KTb{ic}") for ic in range(ND)]
            QTb = [proj.tile([P, L], bf16, tag=f"QTb{ic}", name=f"QTb{ic}") for ic in range(ND)]
            V = [proj.tile([P, D], f32, tag=f"V{jc}", name=f"V{jc}") for jc in range(NL)]

            with tc.tile_pool(name="xw", bufs=1) as xw:
                # ---------------- phase 0: loads ----------------
                xTh = [xw.tile([P, L], bf16, tag=f"xTh{dc}", name=f"xTh{dc}") for dc in range(ND)]
                xTl = [xw.tile([P, L], bf16, tag=f"xTl{dc}", name=f"xTl{dc}") for dc in range(ND)]
                wqh = [xw.tile([P, D], bf16, tag=f"wqh{dc}", name=f"wqh{dc}") for dc in range(ND)]
                wkh = [xw.tile([P, D], bf16, tag=f"wkh{dc}", name=f"wkh{dc}") for dc in range(ND)]
                wkl = [xw.tile([P, D], bf16, tag=f"wkl{dc}", name=f"wkl{dc}") for dc in range(ND)]
                wvh = [xw.tile([P, D], bf16, tag=f"wvh{dc}", name=f"wvh{dc}") for dc in range(ND)]
                wvl = [xw.tile([P, D], bf16, tag=f"wvl{dc}", name=f"wvl{dc}") for dc in range(ND)]
                wqT = [xw.tile([P, D], f32, tag=f"wqT{dc}", name=f"wqT{dc}") for dc in range(ND)]
                wvT = [xw.tile([P, D], f32, tag=f"wvT{dc}", name=f"wvT{dc}") for dc in range(ND)]
                xmT = [xw.tile([P, 1], f32, tag=f"xmT{dc}", name=f"xmT{dc}") for dc in range(ND)]
                for dc in range(ND):
                    sl = slice(dc * P, (dc + 1) * P)
                    nc.sync.dma_start(xTh[dc][:], xth_d[sl, :])
                    nc.sync.dma_start(xTl[dc][:], xtl_d[sl, :])
                    nc.sync.dma_start(wqh[dc][:], wqh_d[sl, :])
                    nc.sync.dma_start(wkh[dc][:], wkh_d[sl, :])
                    nc.sync.dma_start(wkl[dc][:], wkl_d[sl, :])
                    nc.sync.dma_start(wvh[dc][:], wvh_d[sl, :])
                    nc.sync.dma_start(wvl[dc][:], wvl_d[sl, :])
                    nc.sync.dma_start(wqT[dc][:], wq_d[sl, :])
                    nc.sync.dma_start(wvT[dc][:], wv_d[sl, :])
                    nc.sync.dma_start(xmT[dc][:], xm_d[sl, :])

                # ---------------- phase 1: projections ----------------
                # K: 3-term bf16x2 (fp32-class), into KT f32 + KTb bf16
                for ic in range(ND):
                    isl = slice(ic * P, (ic + 1) * P)
                    for jb in range(NJ):
                        jsl = slice(jb * 512, (jb + 1) * 512)
                        pk = ps.tile([P, 512], f32, tag="blk")
                        n = 0
                        for dc in range(ND):
                            for lh, rh in (
                                (wkh[dc][:, isl], xTh[dc][:, jsl]),
                                (wkh[dc][:, isl], xTl[dc][:, jsl]),
                                (wkl[dc][:, isl], xTh[dc][:, jsl]),
                            ):
                                nc.tensor.matmul(
                                    pk[:], lh, rh,
                                    start=(n == 0), stop=(n == 3 * ND - 1),
                                )
                                n += 1
                        nc.scalar.copy(KT[ic][:, jsl], pk[:])
                        nc.vector.tensor_copy(KTb[ic][:, jsl], pk[:])

                # Q approx: single bf16 term
                for ic in range(ND):
                    isl = slice(ic * P, (ic + 1) * P)
                    for jb in range(NJ):
                        jsl = slice(jb * 512, (jb + 1) * 512)
                        pq = ps.tile([P, 512], f32, tag="blk")
                        for dc in range(ND):
                            nc.tensor.matmul(
                                pq[:], wqh[dc][:, isl], xTh[dc][:, jsl],
                                start=(dc == 0), stop=(dc == ND - 1),
                            )
                        nc.scalar.copy(QTb[ic][:, jsl], pq[:])

                # V: 3-term bf16x2
                for jc in range(NL):
                    jsl = slice(jc * P, (jc + 1) * P)
                    pv = ps.tile([P, 512], f32, tag="blk")
                    n = 0
                    for dc in range(ND):
                        for lh, rh in (
                            (xTh[dc][:, jsl], wvh[dc][:]),
                            (xTl[dc][:, jsl], wvh[dc][:]),
                            (xTh[dc][:, jsl], wvl[dc][:]),
                        ):
                            nc.tensor.matmul(
                                pv[:], lh, rh,
                                start=(n == 0), stop=(n == 3 * ND - 1),
                            )
                            n += 1
                    nc.scalar.copy(V[jc][:], pv[:])

                # Vmean = xmeanT.T @ WvT (exact fp32), then broadcast + write ctx
                pvm = ps.tile([1, 512], f32, tag="blk")
                for dc in range(ND):
                    nc.tensor.matmul(
                        pvm[:1, :], xmT[dc][:], wvT[dc][:],
                        start=(dc == 0), stop=(dc == ND - 1),
                    )
                vmean = mst.tile([1, 512], f32, tag="vmean")
                nc.scalar.copy(vmean[:], pvm[:1, :])
                pvb = ps.tile([P, 512], f32, tag="blk")
                nc.tensor.matmul(pvb[:], ones_r1[:], vmean[:], start=True, stop=True)
                vmean_bc = mst.tile([P, 512], f32, tag="vmean_bc")
                nc.vector.tensor_copy(vmean_bc[:], pvb[:])
                for jc in range(NL):
                    nc.sync.dma_start(ctx_d[jc * P : (jc + 1) * P, :], vmean_bc[:])

                # ---------------- phase 2: approx M (bf16 S) ----------------
                # per (lc, jb) block: one STT (masked product -> bf16 scratch,
                # fused sum accum) + one reduce_max. Combines batched at end.
                M_all = mst.tile([P, 16], f32, tag="M_all")
                amax_all = mst.tile([P, NL * NJ], f32, tag="amax_all")
                asum_all = mst.tile([P, NL * NJ], f32, tag="asum_all")
                for lc in range(NL):
                    lsl = slice(lc * P, (lc + 1) * P)
                    mk = mstr.tile([P, L], u8, tag="mk")
                    nc.sync.dma_start(mk[:], mask_d[lsl, :])
                    for jb in range(NJ):
                        jsl = slice(jb * 512, (jb + 1) * 512)
                        k = lc * NJ + jb
                        pss = ps_s.tile([P, 512], f32, tag="psSc", name="pssa")
                        for ic in range(ND):
                            nc.tensor.matmul(
                                pss[:], QTb[ic][:, lsl], KTb[ic][:, jsl],
                                start=(ic == 0), stop=(ic == ND - 1),
                            )
                        s1 = scr.tile([P, 512], bf16, tag="scrt")
                        nc.vector.scalar_tensor_tensor(
                            out=s1[:], in0=pss[:], scalar=1.0, in1=mk[:, jsl],
                            op0=OP.mult, op1=OP.mult,
                            accum_out=asum_all[:, k : k + 1],
                        )
                        nc.vector.reduce_max(
                            amax_all[:, k : k + 1], s1[:], axis=AX.X
                        )
                t1 = accp.tile([P, 16], f32, tag="t1")
                t2 = accp.tile([P, 16], f32, tag="t2")
                nc.vector.reduce_max(
                    t1[:], amax_all[:].rearrange("p (c j) -> p c j", j=NJ),
                    axis=AX.X,
                )
                nc.vector.reduce_sum(
                    t2[:], asum_all[:].rearrange("p (c j) -> p c j", j=NJ),
                    axis=AX.X,
                )
                nc.vector.tensor_scalar_mul(t2[:], t2[:], -1.0 / L)
                nc.vector.tensor_tensor(
                    out=M_all[:], in0=t1[:], in1=t2[:], op=OP.add
                )

                # ---------------- phase 3: approx top-40 -> candidates ------
                pmt = ps.tile([16, P], f32, tag="blk")
                nc.tensor.transpose(pmt[:16, :P], M_all[:], ident[:])
                work = mst.tile([16, P], f32, tag="work")
                nc.vector.tensor_copy(work[:], pmt[:16, :P])
                cand40 = mst.tile([16, NT], f32, tag="cand40")
                for r in range(5):
                    nc.vector.max(out=cand40[:, 8 * r : 8 * r + 8], in_=work[:])
                    if r < 4:
                        nc.vector.match_replace(
                            out=work[:], in_to_replace=cand40[:, 8 * r : 8 * r + 8],
                            in_values=work[:], imm_value=NEG,
                        )
                lin_dram = drp.tile([16 * NT], f32, tag="lin_dram")
                nc.sync.dma_start(lin_dram[:], cand40[:])
                lin = mst.tile([1, 16 * NT], f32, tag="lin")
                nc.sync.dma_start(lin[:], lin_dram[:].rearrange("(a b) -> a b", a=1))
                top40 = mst.tile([1, NT], f32, tag="top40")
                for r in range(5):
                    nc.vector.max(out=top40[:, 8 * r : 8 * r + 8], in_=lin[:])
                    if r < 4:
                        nc.vector.match_replace(
                            out=lin[:], in_to_replace=top40[:, 8 * r : 8 * r + 8],
                            in_values=lin[:], imm_value=NEG,
                        )
                t40d = mst.tile([1, 1], f32, tag="t40d")
                nc.vector.tensor_scalar_add(t40d[:], top40[:, NT - 1 : NT], -DELTA)
                ptb = ps.tile([P, 1], f32, tag="blk")
                nc.tensor.matmul(ptb[:P, :1], ones_r1[:], t40d[:], start=True, stop=True)
                tbc = mst.tile([P, 1], f32, tag="tbc")
                nc.vector.tensor_copy(tbc[:], ptb[:P, :1])

                selmask = mst.tile([P, 16], u8, tag="selmask")
                nc.vector.tensor_tensor(
                    out=selmask[:], in0=M_all[:], in1=tbc[:].to_broadcast([P, 16]),
                    op=OP.is_ge,
                )
                midx = mst.tile([P, 16], f32, tag="midx")
                nc.vector.tensor_copy(midx[:], negone[:].to_broadcast([P, 16]))
                nc.vector.copy_predicated(midx[:], selmask[:], qidx_f[:])

                wrap_in = mst.tile([16, P], f32, tag="wrap_in")
                nc.sync.dma_start(wrap_in[:], midx[:])
                spg = mst.tile([16, 8], f32, tag="spg")
                nfound = mst.tile([1, 1], u32, tag="nfound")
                nc.gpsimd.sparse_gather(out=spg[:], in_=wrap_in[:], num_found=nfound[:])
                spg_cl = mst.tile([16, 8], f32, tag="spg_cl")
                nc.vector.tensor_scalar_max(spg_cl[:], spg[:], 0.0)
                nc.vector.tensor_scalar_min(spg_cl[:], spg_cl[:], float(L - 1))

                cand_dram = drp.tile([P], f32, tag="cand_dram")
                nc.sync.dma_start(
                    cand_dram[:].rearrange("(f p) -> p f", p=16), spg_cl[:]
                )
                candq_f = mst.tile([P, 1], f32, tag="candq_f")
                nc.sync.dma_start(candq_f[:], cand_dram[:].rearrange("l -> l ()"))
                candq_i = mst.tile([P, 1], i32, tag="candq_i")
                nc.vector.tensor_copy(candq_i[:], candq_f[:])

                nf_f = mst.tile([1, 1], f32, tag="nf_f")
                nc.vector.tensor_copy(nf_f[:], nfound[:])
                pnb = ps.tile([P, 1], f32, tag="blk")
                nc.tensor.matmul(pnb[:P, :1], ones_r1[:], nf_f[:], start=True, stop=True)
                nbc = mst.tile([P, 1], f32, tag="nbc")
                nc.vector.tensor_copy(nbc[:], pnb[:P, :1])
                invalid = mst.tile([P, 1], u8, tag="invalid")
                nc.vector.tensor_tensor(
                    out=invalid[:], in0=qidx_f[:, 0:1], in1=nbc[:], op=OP.is_ge
                )

                # ---------------- phase 4a: exact candidates ----------------
                x_cand = cnd.tile([P, D], f32, tag="x_cand")
                nc.gpsimd.indirect_dma_start(
                    out=x_cand[:], out_offset=None, in_=x_d[:],
                    in_offset=bass.IndirectOffsetOnAxis(ap=candq_i[:, :1], axis=0),
                )
                xcT = [cnd.tile([P, P], f32, tag=f"xcT{dc}", name=f"xcT{dc}") for dc in range(ND)]
                for dc in range(ND):
                    pxc = ps.tile([P, P], f32, tag="blk")
                    nc.tensor.transpose(
                        pxc[:P, :P], x_cand[:, dc * P : (dc + 1) * P], ident[:]
                    )
                    nc.vector.tensor_copy(xcT[dc][:], pxc[:P, :P])

                QcT = [cnd.tile([P, P], f32, tag=f"QcT{ic}", name=f"QcT{ic}") for ic in range(ND)]
                for ic in range(ND):
                    isl = slice(ic * P, (ic + 1) * P)
                    pqc = ps.tile([P, P], f32, tag="blk")
                    for dc in range(ND):
                        nc.tensor.matmul(
                            pqc[:P, :P], wqT[dc][:, isl], xcT[dc][:],
                            start=(dc == 0), stop=(dc == ND - 1),
                        )
                    nc.vector.tensor_copy(QcT[ic][:], pqc[:P, :P])

                gm = cnd.tile([P, L], u8, tag="gm")
                nc.gpsimd.indirect_dma_start(
                    out=gm[:], out_offset=None, in_=mask_d[:],
                    in_offset=bass.IndirectOffsetOnAxis(ap=candq_i[:, :1], axis=0),
                )
                gc = cnd.tile([P, L], u8, tag="gc")
                nc.gpsimd.indirect_dma_start(
                    out=gc[:], out_offset=None, in_=cnt_d[:],
                    in_offset=bass.IndirectOffsetOnAxis(ap=candq_i[:, :1], axis=0),
                )

                psS = []
                cmax = cnd.tile([P, NJ], f32, tag="cmax")
                csum = cnd.tile([P, NJ], f32, tag="csum")
                for jb in range(NJ):
                    jsl = slice(jb * 512, (jb + 1) * 512)
                    pss2 = ps_s.tile([P, 512], f32, tag="psSc")
                    psS.append(pss2)
                    for ic in range(ND):
                        nc.tensor.matmul(
                            pss2[:], QcT[ic][:], KT[ic][:, jsl],
                            start=(ic == 0), stop=(ic == ND - 1),
                        )
                    s3 = scr.tile([P, 512], f32, tag="scrt")
                    nc.vector.tensor_tensor(
                        out=s3[:], in0=pss2[:], in1=gm[:, jsl], op=OP.mult
                    )
                    nc.vector.reduce_max(cmax[:, jb : jb + 1], s3[:], axis=AX.X)
                    s4 = scr.tile([P, 512], f32, tag="scrt")
                    nc.vector.scalar_tensor_tensor(
                        out=s4[:], in0=pss2[:], scalar=-1.0 / L, in1=gc[:, jsl],
                        op0=OP.mult, op1=OP.mult,
                        accum_out=csum[:, jb : jb + 1],
                    )
                u1 = cnd.tile([P, 1], f32, tag="u1")
                u2 = cnd.tile([P, 1], f32, tag="u2")
                M_cand = cnd.tile([P, 1], f32, tag="M_cand")
                nc.vector.reduce_max(u1[:], cmax[:], axis=AX.X)
                nc.vector.reduce_sum(u2[:], csum[:], axis=AX.X)
                nc.vector.tensor_tensor(out=M_cand[:], in0=u1[:], in1=u2[:], op=OP.add)
                nc.vector.copy_predicated(M_cand[:], invalid[:], negbig[:])

                # exact top-40 threshold among candidates
                pmc = ps.tile([1, P], f32, tag="blk")
                nc.tensor.transpose(pmc[:1, :P], M_cand[:], ident[:])
                mcT = cnd.tile([1, P], f32, tag="mcT")
                nc.vector.tensor_copy(mcT[:], pmc[:1, :P])
                etop = cnd.tile([1, NT], f32, tag="etop")
                for r in range(5):
                    nc.vector.max(out=etop[:, 8 * r : 8 * r + 8], in_=mcT[:])
                    if r < 4:
                        nc.vector.match_replace(
                            out=mcT[:], in_to_replace=etop[:, 8 * r : 8 * r + 8],
                            in_values=mcT[:], imm_value=NEG,
                        )
                pte = ps.tile([P, 1], f32, tag="blk")
                nc.tensor.matmul(
                    pte[:P, :1], ones_r1[:], etop[:, NT - 1 : NT], start=True, stop=True
                )
                tebc = cnd.tile([P, 1], f32, tag="tebc")
                nc.vector.tensor_copy(tebc[:], pte[:P, :1])
                sel2 = cnd.tile([P, 1], u8, tag="sel2")
                nc.vector.tensor_tensor(
                    out=sel2[:], in0=M_cand[:], in1=tebc[:], op=OP.is_ge
                )
                scat_f = cnd.tile([P, 1], f32, tag="scat_f")
                nc.vector.tensor_copy(scat_f[:], big9[:])
                nc.vector.copy_predicated(scat_f[:], sel2[:], candq_f[:])
                scat_i = cnd.tile([P, 1], i32, tag="scat_i")
                nc.vector.tensor_copy(scat_i[:], scat_f[:])

            # xTh/xTl/weights freed here
            with tc.tile_pool(name="expp", bufs=1) as expp:
                # ---------------- phase 4b: softmax + update ----------------
                rmax4 = expp.tile([P, NJ], f32, tag="rmax4")
                for jb in range(NJ):
                    nc.vector.reduce_max(rmax4[:, jb : jb + 1], psS[jb][:], axis=AX.X)
                rmax = expp.tile([P, 1], f32, tag="rmax")
                nc.vector.reduce_max(rmax[:], rmax4[:], axis=AX.X)
                negb = expp.tile([P, 1], f32, tag="negb")
                nc.vector.tensor_scalar_mul(negb[:], rmax[:], -SCALE)
                exp_sb = expp.tile([P, L], f32, tag="exp_sb")
                sume4 = expp.tile([P, NJ], f32, tag="sume4")
                for jb in range(NJ):
                    jsl = slice(jb * 512, (jb + 1) * 512)
                    nc.scalar.activation(
                        out=exp_sb[:, jsl], in_=psS[jb][:], func=ACTF.Exp,
                        bias=negb[:], scale=SCALE,
                        accum_out=sume4[:, jb : jb + 1],
                    )
                sume = expp.tile([P, 1], f32, tag="sume")
                nc.vector.reduce_sum(sume[:], sume4[:], axis=AX.X)
                recip = expp.tile([P, 1], f32, tag="recip")
                nc.vector.reciprocal(recip[:], sume[:])

                expT = [expp.tile([P, P], f32, tag=f"expT{jc}", name=f"expT{jc}") for jc in range(NL)]
                for jc in range(NL):
                    pet = ps.tile([P, P], f32, tag="blk")
                    nc.tensor.transpose(
                        pet[:P, :P], exp_sb[:, jc * P : (jc + 1) * P], ident[:]
                    )
                    nc.vector.tensor_copy(expT[jc][:], pet[:P, :P])

                pu = ps.tile([P, 512], f32, tag="blk")
                for jc in range(NL):
                    nc.tensor.matmul(
                        pu[:], expT[jc][:], V[jc][:],
                        start=(jc == 0), stop=(jc == NL - 1),
                    )
                upd = expp.tile([P, D], f32, tag="upd")
                nc.scalar.activation(
                    out=upd[:], in_=pu[:], func=ACTF.Copy, bias=0.0, scale=recip[:]
                )
                nc.gpsimd.indirect_dma_start(
                    out=ctx_d[:],
                    out_offset=bass.IndirectOffsetOnAxis(ap=scat_i[:, :1], axis=0),
                    in_=upd[:], in_offset=None,
                    bounds_check=L - 1, oob_is_err=False,
                )

    nc.compile()
    return nc


_NC = None


def _get_nc():
    global _NC
    if _NC is None:
        _NC = build()
    return _NC


def _split_bf16(a):
    hi = a.astype(ml_dtypes.bfloat16)
    lo = (a - hi.astype(np.float32)).astype(ml_dtypes.bfloat16)
    return hi, lo


def _host_prep(x, Wq, Wk, Wv, index_sample):
    x = np.asarray(x, dtype=np.float32)
    Wq = np.asarray(Wq, dtype=np.float32)
    Wk = np.asarray(Wk, dtype=np.float32)
    Wv = np.asarray(Wv, dtype=np.float32)
    idx = np.asarray(index_sample)

    wqT = np.ascontiguousarray(Wq.T)
    wvT = np.ascontiguousarray(Wv.T)
    wqh, _ = _split_bf16(wqT)
    wkh, wkl = _split_bf16(np.ascontiguousarray(Wk.T))
    wvh, wvl = _split_bf16(wvT)

    rows = np.arange(L)[:, None]
    mask01 = np.zeros((L, L), dtype=np.uint8)
    mask01[rows, idx] = 1
    countf = np.zeros((L, L), dtype=np.uint8)
    np.add.at(countf, (rows, idx), 1)

    shared = {
        "wqTh": wqh, "wkTh": wkh, "wkTl": wkl, "wvTh": wvh, "wvTl": wvl,
        "wqT": wqT, "wvT": wvT, "mask01": mask01, "countf": countf,
    }
    in_maps = []
    for b in range(B):
        xb = np.ascontiguousarray(x[b])
        xT = np.ascontiguousarray(xb.T)
        xth, xtl = _split_bf16(xT)
        xmean = (xb.astype(np.float64).mean(axis=0) / 1.0).astype(np.float32)
        in_maps.append(
            {
                "x_nat": xb,
                "xTh": xth,
                "xTl": xtl,
                "xmeanT": xmean.reshape(D, 1),
                **shared,
            }
        )
    return in_maps


def kernel(x, Wq, Wk, Wv, index_sample, _trace=False, _result_box=None):
    in_maps = _host_prep(x, Wq, Wk, Wv, index_sample)
    nc = _get_nc()
    res = run_bass_kernel_spmd(nc, in_maps, core_ids=list(range(B)), trace=_trace)
    if _result_box is not None:
        _result_box.append(res)
    out = np.stack([np.asarray(res.results[b]["ctx"]) for b in range(B)], axis=0)
    return out
